# revision 12
# baseline (speedup 1.0000x reference)
"""Causal self-attention (B=2, S=2048, D=1024, H=16) on 8 Trainium2 cores.

Sharding: batch x head-group. Core c handles batch c//4 and heads
[4*(c%4), 4*(c%4)+4). Each core computes q/k/v projections for its head
slice, causal flash-attention (transposed layout, no max-subtraction --
scores are bounded ~9), and a row-parallel partial output projection.
The host transposes/sums the 8 partial outputs and adds b_proj.

All matmuls run in float32r (~1.5 cyc/row on the PE for free dim >=256).
"""

import sys

import numpy as np

try:
    import concourse.bass as bass  # noqa: F401
except ImportError:  # fallback for environments without the site hook
    sys.path.insert(0, "/opt/trn_rl_repo")

import concourse.bacc as bacc
import concourse.bass as bass
import concourse.mybir as mybir
from concourse import tile
from concourse.bass_utils import run_bass_kernel_spmd

B, S, D, H = 2, 2048, 1024, 16
HD = D // H  # 64
SCALE = 1.0 / np.sqrt(HD)  # 0.125
HPC = 4          # heads per core
NCORES = 8
P = 128          # partitions
QC = 512         # query chunk (matmul free dim)
NQ = S // QC     # 4 query chunks
NK = S // P      # 16 key tiles
ND = D // P      # 8 d tiles
F32 = mybir.dt.float32
F32R = mybir.dt.float32r
BF16 = mybir.dt.bfloat16
ATT_BF16 = True                 # scores + attn@v in bf16 (1 cyc/row + FWL)
ATT_DT = BF16 if ATT_BF16 else F32R
VPAD = 336                      # v tile cols: 4*65 rounded up so every
                                # head slice can read a full 128-col lhsT

_PROGRAM = None


def _build_program():
    """Build the SPMD Bass program (same NEFF for all 8 cores)."""
    nc = bacc.Bacc(None, target_bir_lowering=False)

    xt = nc.declare_dram_parameter("xt", [D + 1, S], F32R, isOutput=False)
    wqk = nc.declare_dram_parameter("wqk", [D, 4 * P], F32R, isOutput=False)
    wv = nc.declare_dram_parameter("wv", [D + 1, HPC * (HD + 1)], F32R, isOutput=False)
    bqk = nc.declare_dram_parameter("bqk", [P, 4], F32, isOutput=False)
    masks = nc.declare_dram_parameter("masks", [P, 8 * QC], ATT_DT, isOutput=False)
    wp = nc.declare_dram_parameter("wp", [HPC * HD, D], F32R, isOutput=False)
    yt = nc.declare_dram_parameter("yt", [D, S], F32, isOutput=True)

    VW = HPC * (HD + 1)  # 260 cols of augmented v

    with tile.TileContext(nc) as tc:
        with (
            tc.tile_pool(name="const", bufs=1) as const,
            tc.tile_pool(name="big", bufs=1) as bigp,
            tc.tile_pool(name="ps_mm", bufs=2, space="PSUM") as ps_mm,
            tc.tile_pool(name="ps_pv", bufs=4, space="PSUM") as ps_pv,
        ):
            xtp_cm = tc.tile_pool(name="xtp", bufs=1)
            xtp = xtp_cm.__enter__()

            # ---- interleave weight + first-chunk x loads so the very
            # first matmul (wqk0 x xt0[sc0]) starts after ~0.75MB of DMA ----
            xt_sb = [
                xtp.tile([P, S], F32R, tag=f"xt{dt}", name=f"xts{dt}")
                for dt in range(ND)
            ]
            wqk_sb = []
            for dt in range(ND):
                t = const.tile([P, 4 * P], F32R, tag=f"wqk{dt}", name=f"wqk{dt}")
                nc.sync.dma_start(t[:], wqk[dt * P:(dt + 1) * P, :])
                wqk_sb.append(t)
                nc.sync.dma_start(
                    xt_sb[dt][:, 0:QC], xt[dt * P:(dt + 1) * P, 0:QC]
                )
            bqk_sb = const.tile([P, 4], F32, tag="bqk")
            nc.sync.dma_start(bqk_sb[:], bqk[:])
            xa_sb = const.tile([1, S], F32R, tag="xa")  # ones row
            nc.sync.dma_start(xa_sb[:], xt[D:D + 1, :])
            for sc in range(1, NQ):
                for dt in range(ND):
                    nc.sync.dma_start(
                        xt_sb[dt][:, sc * QC:(sc + 1) * QC],
                        xt[dt * P:(dt + 1) * P, sc * QC:(sc + 1) * QC],
                    )

            wv_sb = []
            for dt in range(ND):
                t = const.tile([P, VW], F32R, tag=f"wv{dt}", name=f"wv{dt}")
                nc.sync.dma_start(t[:], wv[dt * P:(dt + 1) * P, :])
                wv_sb.append(t)
            wva_sb = const.tile([1, VW], F32R, tag="wva")  # bias+ones row
            nc.sync.dma_start(wva_sb[:], wv[D:D + 1, :])

            masks_sb = const.tile([P, 8 * QC], ATT_DT, tag="masks")
            nc.sync.dma_start(masks_sb[:], masks[:])
            wp_sb = []
            for i in range(2):
                t = const.tile([P, D], F32R, tag=f"wp{i}", name=f"wp{i}")
                nc.sync.dma_start(t[:], wp[i * P:(i + 1) * P, :])
                wp_sb.append(t)

            # ---- persistent intermediates ----
            qt_sb = [bigp.tile([P, S], ATT_DT, tag=f"qt{i}", name=f"qt{i}") for i in range(2)]
            kt_sb = [bigp.tile([P, S], ATT_DT, tag=f"kt{i}", name=f"kt{i}") for i in range(2)]
            v_sb = [bigp.tile([P, VPAD], ATT_DT, tag=f"v{i}", name=f"v{i}") for i in range(NK)]
            ot_sb = [bigp.tile([P, S], F32R, tag=f"ot{i}", name=f"ot{i}") for i in range(2)]

            # ================= phase 1: q/k projections =================
            for sc in range(NQ):
                for et in range(4):  # 0,1: q heads (0,1),(2,3); 2,3: k heads
                    ps = ps_mm.tile([P, QC], F32, tag="mm", name=f"qk{sc}{et}")
                    for dt in range(ND):
                        nc.tensor.matmul(
                            ps[:],
                            wqk_sb[dt][:, et * P:(et + 1) * P],
                            xt_sb[dt][:, sc * QC:(sc + 1) * QC],
                            start=(dt == 0),
                            stop=(dt == ND - 1),
                        )
                    dest = (qt_sb if et < 2 else kt_sb)[et % 2]
                    dst_ap = dest[:, sc * QC:(sc + 1) * QC]
                    if et < 2:
                        nc.scalar.activation(
                            dst_ap, ps[:],
                            mybir.ActivationFunctionType.Identity,
                            bias=bqk_sb[:, et:et + 1],
                        )
                    else:
                        nc.vector.tensor_scalar_add(dst_ap, ps[:], bqk_sb[:, et:et + 1])

            # ================= phase 1b: v projection =================
            def emit_v(st):
                ps = ps_mm.tile([P, VW], F32, tag="mm", name=f"vp{st}")
                for dt in range(ND):
                    nc.tensor.matmul(
                        ps[:],
                        xt_sb[dt][:, st * P:(st + 1) * P],
                        wv_sb[dt][:],
                        start=(dt == 0),
                        stop=False,
                    )
                nc.tensor.matmul(  # bias + ones column via rank-1 update
                    ps[:],
                    xa_sb[:, st * P:(st + 1) * P],
                    wva_sb[:],
                    start=False,
                    stop=True,
                )
                if st % 2 == 0:
                    nc.scalar.copy(v_sb[st][:, 0:VW], ps[:])
                else:
                    nc.vector.tensor_copy(v_sb[st][:, 0:VW], ps[:])

            for st in range(4):
                emit_v(st)

            work_cm = tc.tile_pool(name="work", bufs=6)
            work = work_cm.__enter__()
            small_cm = tc.tile_pool(name="small", bufs=3)
            small = small_cm.__enter__()

            # ================= phase 2: attention =================
            def emit_pair(qt, pair):
                q0 = qt * QC
                nk = (qt + 1) * (QC // P)  # causal: k tiles 0..nk-1
                ht = pair
                pvs = [
                    ps_pv.tile([P, QC], F32, tag="pv", name=f"pv{qt}{pair}{hh}")
                    for hh in range(2)
                ]
                for kb in range(nk):
                    j = kb - qt * (QC // P)
                    # diagonal strip: columns < 128*j are fully masked;
                    # shrink N (keep >=256 for full matmul rate)
                    off = 0 if j < 0 else min(P * j, QC - 256)
                    w = QC - off
                    st2 = ps_mm.tile(
                        [P, 2 * QC], F32, tag="mm", name=f"st{qt}{pair}{kb}"
                    )
                    for hh in range(2):
                        nc.tensor.matmul(
                            st2[:, hh * QC + off:(hh + 1) * QC],
                            kt_sb[ht][slice(64 * hh, 64 * hh + 64),
                                      kb * P:(kb + 1) * P],
                            qt_sb[ht][slice(64 * hh, 64 * hh + 64),
                                      q0 + off:q0 + QC],
                            start=True, stop=True,
                            tile_position=(64 * hh, 0),
                        )
                    ex = work.tile(
                        [P, 2 * QC], ATT_DT, tag="ex", name=f"ex{qt}{pair}{kb}"
                    )
                    st3 = st2[:].rearrange("p (h q) -> p h q", h=2)[:, :, off:]
                    ex3 = ex[:].rearrange("p (h q) -> p h q", h=2)[:, :, off:]
                    nc.scalar.activation(
                        ex3, st3,
                        mybir.ActivationFunctionType.Exp,
                        scale=float(SCALE),
                    )
                    if j >= 0:
                        m3 = masks_sb[:, 2 * j * QC:2 * (j + 1) * QC].rearrange(
                            "p (h q) -> p h q", h=2)[:, :, off:]
                        nc.vector.tensor_mul(ex3, ex3, m3)
                    for hh in range(2):
                        h = 2 * pair + hh
                        nc.tensor.matmul(
                            pvs[hh][:, off:],
                            v_sb[kb][:, h * (HD + 1):h * (HD + 1) + P],
                            ex[:, hh * QC + off:(hh + 1) * QC],
                            start=(kb == 0),
                            stop=(kb == nk - 1),
                        )
                for hh in range(2):
                    # rows 0..63 are o^T, row 64 is the denominator
                    # (reciprocal_approx_fast misreads PSUM -> copy first)
                    dcp = small.tile(
                        [1, QC], F32, tag="dcp", name=f"dcp{qt}{pair}{hh}"
                    )
                    nc.vector.tensor_copy(dcp[:], pvs[hh][HD:HD + 1, :])
                    rden = small.tile(
                        [1, QC], F32, tag="rden", name=f"rden{qt}{pair}{hh}"
                    )
                    nc.vector.reciprocal_approx_fast(rden[:], dcp[:])
                    bden = small.tile(
                        [64, QC], F32, tag="bden", name=f"bden{qt}{pair}{hh}"
                    )
                    nc.gpsimd.partition_broadcast(bden[:], rden[:])
                    nc.vector.tensor_mul(
                        ot_sb[ht][slice(64 * hh, 64 * hh + 64), q0:q0 + QC],
                        pvs[hh][0:HD, :], bden[:],
                    )

            def emit_proj(qt):
                q0 = qt * QC
                for et in range(8):
                    ps = ps_pv.tile([P, QC], F32, tag="pv", name=f"yp{qt}{et}")
                    for i in range(2):
                        nc.tensor.matmul(
                            ps[:],
                            wp_sb[i][:, et * P:(et + 1) * P],
                            ot_sb[i][:, q0:q0 + QC],
                            start=(i == 0),
                            stop=(i == 1),
                        )
                    ystage = small.tile([P, QC], F32, tag="ys", name=f"ys{qt}{et}")
                    if et % 2 == 0:
                        nc.scalar.copy(ystage[:], ps[:])
                    else:
                        nc.vector.tensor_copy(ystage[:], ps[:])
                    nc.sync.dma_start(yt[et * P:(et + 1) * P, q0:q0 + QC], ystage[:])

            # software-pipelined emission: proj(qt) goes into the middle of
            # attention(qt+1) so the PE stream never head-of-line blocks on
            # the DVE normalize chain; V tiles trickle in between.
            emit_pair(0, 0)
            for st in range(4, 8):
                emit_v(st)
            emit_pair(0, 1)
            for st in range(8, 12):
                emit_v(st)
            emit_pair(1, 0)
            emit_proj(0)
            for st in range(12, 16):
                emit_v(st)
            emit_pair(1, 1)
            emit_pair(2, 0)
            emit_proj(1)
            emit_pair(2, 1)
            emit_pair(3, 0)
            emit_proj(2)
            emit_pair(3, 1)
            emit_proj(3)

            small_cm.__exit__(None, None, None)
            work_cm.__exit__(None, None, None)
            xtp_cm.__exit__(None, None, None)

    nc.compile()
    return nc


def _shard_inputs(x, w_qkv, b_qkv, w_proj):
    """Build the per-core input maps."""
    in_maps = []
    ones_row = np.ones((1, S), np.float32)
    kk = np.arange(P)[:, None]
    qq = np.arange(QC)[None, :]
    if ATT_BF16:
        import ml_dtypes
        mdt = ml_dtypes.bfloat16
    else:
        mdt = np.float32
    masks_np = np.concatenate(
        sum([[(qq >= kk + P * j).astype(mdt)] * 2 for j in range(4)], []), axis=1
    )
    for c in range(NCORES):
        b, g = divmod(c, 4)
        e0 = g * HPC * HD  # 256*g
        xt_np = np.concatenate([np.ascontiguousarray(x[b].T), ones_row], 0)
        q_rows = w_qkv[e0:e0 + HPC * HD]            # [256, 1024]
        k_rows = w_qkv[D + e0:D + e0 + HPC * HD]
        wqk_np = np.concatenate([q_rows.T, k_rows.T], 1)  # [1024, 512]
        wv_np = np.zeros((D + 1, HPC * (HD + 1)), np.float32)
        for h in range(HPC):
            rows = 2 * D + e0 + h * HD
            wv_np[:D, h * (HD + 1):h * (HD + 1) + HD] = w_qkv[rows:rows + HD].T
            wv_np[D, h * (HD + 1):h * (HD + 1) + HD] = b_qkv[rows:rows + HD]
            wv_np[D, h * (HD + 1) + HD] = 1.0
        bqk_np = np.stack(
            [b_qkv[e0:e0 + P], b_qkv[e0 + P:e0 + 2 * P],
             b_qkv[D + e0:D + e0 + P], b_qkv[D + e0 + P:D + e0 + 2 * P]], 1
        ).astype(np.float32)
        wp_np = np.ascontiguousarray(w_proj[:, e0:e0 + HPC * HD].T)  # [256, 1024]
        in_maps.append({
            "xt": np.ascontiguousarray(xt_np, np.float32),
            "wqk": np.ascontiguousarray(wqk_np, np.float32),
            "wv": wv_np,
            "bqk": np.ascontiguousarray(bqk_np),
            "masks": masks_np,
            "wp": wp_np,
        })
    return in_maps


def _run(inputs, trace=False, trace_kwargs=None):
    global _PROGRAM
    if _PROGRAM is None:
        _PROGRAM = _build_program()
    nc = _PROGRAM
    x = np.asarray(inputs["x"], np.float32)
    w_qkv = np.asarray(inputs["w_qkv"], np.float32)
    b_qkv = np.asarray(inputs["b_qkv"], np.float32)
    w_proj = np.asarray(inputs["w_proj"], np.float32)
    b_proj = np.asarray(inputs["b_proj"], np.float32)
    in_maps = _shard_inputs(x, w_qkv, b_qkv, w_proj)
    res = run_bass_kernel_spmd(
        nc, in_maps, core_ids=list(range(NCORES)),
        trace=trace, **(trace_kwargs or {}),
    )
    y = np.zeros((B, S, D), np.float32)
    for c in range(NCORES):
        y[c // 4] += res.results[c]["yt"].T
    y += b_proj
    return y, res


def kernel(**inputs):
    y, _ = _run(inputs)
    return y


# revision 14
# speedup vs baseline: 1.0800x; 1.0800x over previous
"""Causal self-attention (B=2, S=2048, D=1024, H=16) on 8 Trainium2 cores.

Sharding: batch x head-group. Core c handles batch c//4 and heads
[4*(c%4), 4*(c%4)+4). Each core computes q/k/v projections for its head
slice, causal flash-attention (transposed layout, no max-subtraction --
scores are bounded ~9), and a row-parallel partial output projection.
The host transposes/sums the 8 partial outputs and adds b_proj.

All matmuls run in float32r (~1.5 cyc/row on the PE for free dim >=256).
"""

import sys

import numpy as np

try:
    import concourse.bass as bass  # noqa: F401
except ImportError:  # fallback for environments without the site hook
    sys.path.insert(0, "/opt/trn_rl_repo")

import concourse.bacc as bacc
import concourse.bass as bass
import concourse.mybir as mybir
from concourse import tile
from concourse.bass_utils import run_bass_kernel_spmd

B, S, D, H = 2, 2048, 1024, 16
HD = D // H  # 64
SCALE = 1.0 / np.sqrt(HD)  # 0.125
HPC = 4          # heads per core
NCORES = 8
P = 128          # partitions
QC = 512         # query chunk (matmul free dim)
NQ = S // QC     # 4 query chunks
NK = S // P      # 16 key tiles
ND = D // P      # 8 d tiles
F32 = mybir.dt.float32
F32R = mybir.dt.float32r
BF16 = mybir.dt.bfloat16
ATT_BF16 = True                 # scores + attn@v in bf16 (1 cyc/row + FWL)
ATT_DT = BF16 if ATT_BF16 else F32R
VPAD = 336                      # v tile cols: 4*65 rounded up so every
                                # head slice can read a full 128-col lhsT

_PROGRAM = None


def _build_program():
    """Build the SPMD Bass program (same NEFF for all 8 cores)."""
    nc = bacc.Bacc(None, target_bir_lowering=False)

    xt = nc.declare_dram_parameter("xt", [D + 1, S], ATT_DT, isOutput=False)
    wqk = nc.declare_dram_parameter("wqk", [D, 4 * P], ATT_DT, isOutput=False)
    wv = nc.declare_dram_parameter("wv", [D + 1, HPC * (HD + 1)], ATT_DT, isOutput=False)
    bqk = nc.declare_dram_parameter("bqk", [P, 4], F32, isOutput=False)
    masks = nc.declare_dram_parameter("masks", [P, 8 * QC], ATT_DT, isOutput=False)
    wp = nc.declare_dram_parameter("wp", [HPC * HD, D], F32R, isOutput=False)
    yt = nc.declare_dram_parameter("yt", [D, S], F32, isOutput=True)

    VW = HPC * (HD + 1)  # 260 cols of augmented v

    with tile.TileContext(nc) as tc:
        with (
            tc.tile_pool(name="const", bufs=1) as const,
            tc.tile_pool(name="big", bufs=1) as bigp,
            tc.tile_pool(name="ps_mm", bufs=2, space="PSUM") as ps_mm,
            tc.tile_pool(name="ps_pv", bufs=4, space="PSUM") as ps_pv,
        ):
            xtp_cm = tc.tile_pool(name="xtp", bufs=1)
            xtp = xtp_cm.__enter__()

            # ---- weights first (small) so compute can start early ----
            wqk_sb = []
            for dt in range(ND):
                t = const.tile([P, 4 * P], ATT_DT, tag=f"wqk{dt}", name=f"wqk{dt}")
                nc.sync.dma_start(t[:], wqk[dt * P:(dt + 1) * P, :])
                wqk_sb.append(t)
            bqk_sb = const.tile([P, 4], F32, tag="bqk")
            nc.sync.dma_start(bqk_sb[:], bqk[:])
            xa_sb = const.tile([1, S], ATT_DT, tag="xa")  # ones row
            nc.sync.dma_start(xa_sb[:], xt[D:D + 1, :])

            # ---- x^T, loaded in s-chunk-major order ----
            xt_sb = [
                xtp.tile([P, S], ATT_DT, tag=f"xt{dt}", name=f"xts{dt}")
                for dt in range(ND)
            ]
            for sc in range(NQ):
                for dt in range(ND):
                    nc.sync.dma_start(
                        xt_sb[dt][:, sc * QC:(sc + 1) * QC],
                        xt[dt * P:(dt + 1) * P, sc * QC:(sc + 1) * QC],
                    )

            wv_sb = []
            for dt in range(ND):
                t = const.tile([P, VW], ATT_DT, tag=f"wv{dt}", name=f"wv{dt}")
                nc.sync.dma_start(t[:], wv[dt * P:(dt + 1) * P, :])
                wv_sb.append(t)
            wva_sb = const.tile([1, VW], ATT_DT, tag="wva")  # bias+ones row
            nc.sync.dma_start(wva_sb[:], wv[D:D + 1, :])

            masks_sb = const.tile([P, 8 * QC], ATT_DT, tag="masks")
            nc.sync.dma_start(masks_sb[:], masks[:])
            wp_sb = []
            for i in range(2):
                t = const.tile([P, D], F32R, tag=f"wp{i}", name=f"wp{i}")
                nc.sync.dma_start(t[:], wp[i * P:(i + 1) * P, :])
                wp_sb.append(t)

            # ---- persistent intermediates ----
            qt_sb = [bigp.tile([P, S], ATT_DT, tag=f"qt{i}", name=f"qt{i}") for i in range(2)]
            kt_sb = [bigp.tile([P, S], ATT_DT, tag=f"kt{i}", name=f"kt{i}") for i in range(2)]
            v_sb = [bigp.tile([P, VPAD], ATT_DT, tag=f"v{i}", name=f"v{i}") for i in range(NK)]
            ot_sb = [bigp.tile([P, S], F32R, tag=f"ot{i}", name=f"ot{i}") for i in range(2)]

            # ================= phase 1: q/k projections =================
            for sc in range(NQ):
                for et in range(4):  # 0,1: q heads (0,1),(2,3); 2,3: k heads
                    ps = ps_mm.tile([P, QC], F32, tag="mm", name=f"qk{sc}{et}")
                    for dt in range(ND):
                        nc.tensor.matmul(
                            ps[:],
                            wqk_sb[dt][:, et * P:(et + 1) * P],
                            xt_sb[dt][:, sc * QC:(sc + 1) * QC],
                            start=(dt == 0),
                            stop=(dt == ND - 1),
                        )
                    dest = (qt_sb if et < 2 else kt_sb)[et % 2]
                    dst_ap = dest[:, sc * QC:(sc + 1) * QC]
                    if et < 2:
                        nc.scalar.activation(
                            dst_ap, ps[:],
                            mybir.ActivationFunctionType.Identity,
                            bias=bqk_sb[:, et:et + 1],
                        )
                    else:
                        nc.vector.tensor_scalar_add(dst_ap, ps[:], bqk_sb[:, et:et + 1])

            # ================= phase 1b: v projection =================
            def emit_v(st):
                ps = ps_mm.tile([P, VW], F32, tag="mm", name=f"vp{st}")
                for dt in range(ND):
                    nc.tensor.matmul(
                        ps[:],
                        xt_sb[dt][:, st * P:(st + 1) * P],
                        wv_sb[dt][:],
                        start=(dt == 0),
                        stop=False,
                    )
                nc.tensor.matmul(  # bias + ones column via rank-1 update
                    ps[:],
                    xa_sb[:, st * P:(st + 1) * P],
                    wva_sb[:],
                    start=False,
                    stop=True,
                )
                if st % 2 == 0:
                    nc.scalar.copy(v_sb[st][:, 0:VW], ps[:])
                else:
                    nc.vector.tensor_copy(v_sb[st][:, 0:VW], ps[:])

            for st in range(4):
                emit_v(st)

            work_cm = tc.tile_pool(name="work", bufs=6)
            work = work_cm.__enter__()
            small_cm = tc.tile_pool(name="small", bufs=3)
            small = small_cm.__enter__()

            # ================= phase 2: attention =================
            def emit_pair(qt, pair):
                q0 = qt * QC
                nk = (qt + 1) * (QC // P)  # causal: k tiles 0..nk-1
                ht = pair
                pvs = [
                    ps_pv.tile([P, QC], F32, tag="pv", name=f"pv{qt}{pair}{hh}")
                    for hh in range(2)
                ]
                for kb in range(nk):
                    j = kb - qt * (QC // P)
                    # diagonal strip: columns < 128*j are fully masked;
                    # shrink N (keep >=256 for full matmul rate)
                    off = 0 if j < 0 else min(P * j, QC - 256)
                    w = QC - off
                    st2 = ps_mm.tile(
                        [P, 2 * QC], F32, tag="mm", name=f"st{qt}{pair}{kb}"
                    )
                    for hh in range(2):
                        nc.tensor.matmul(
                            st2[:, hh * QC + off:(hh + 1) * QC],
                            kt_sb[ht][slice(64 * hh, 64 * hh + 64),
                                      kb * P:(kb + 1) * P],
                            qt_sb[ht][slice(64 * hh, 64 * hh + 64),
                                      q0 + off:q0 + QC],
                            start=True, stop=True,
                            tile_position=(64 * hh, 0),
                        )
                    ex = work.tile(
                        [P, 2 * QC], ATT_DT, tag="ex", name=f"ex{qt}{pair}{kb}"
                    )
                    st3 = st2[:].rearrange("p (h q) -> p h q", h=2)[:, :, off:]
                    ex3 = ex[:].rearrange("p (h q) -> p h q", h=2)[:, :, off:]
                    nc.scalar.activation(
                        ex3, st3,
                        mybir.ActivationFunctionType.Exp,
                        scale=float(SCALE),
                    )
                    if j >= 0:
                        m3 = masks_sb[:, 2 * j * QC:2 * (j + 1) * QC].rearrange(
                            "p (h q) -> p h q", h=2)[:, :, off:]
                        nc.vector.tensor_mul(ex3, ex3, m3)
                    for hh in range(2):
                        h = 2 * pair + hh
                        nc.tensor.matmul(
                            pvs[hh][:, off:],
                            v_sb[kb][:, h * (HD + 1):h * (HD + 1) + P],
                            ex[:, hh * QC + off:(hh + 1) * QC],
                            start=(kb == 0),
                            stop=(kb == nk - 1),
                        )
                for hh in range(2):
                    # rows 0..63 are o^T, row 64 is the denominator
                    # (reciprocal_approx_fast misreads PSUM -> copy first)
                    dcp = small.tile(
                        [1, QC], F32, tag="dcp", name=f"dcp{qt}{pair}{hh}"
                    )
                    nc.vector.tensor_copy(dcp[:], pvs[hh][HD:HD + 1, :])
                    rden = small.tile(
                        [1, QC], F32, tag="rden", name=f"rden{qt}{pair}{hh}"
                    )
                    nc.vector.reciprocal_approx_fast(rden[:], dcp[:])
                    bden = small.tile(
                        [64, QC], F32, tag="bden", name=f"bden{qt}{pair}{hh}"
                    )
                    nc.gpsimd.partition_broadcast(bden[:], rden[:])
                    nc.vector.tensor_mul(
                        ot_sb[ht][slice(64 * hh, 64 * hh + 64), q0:q0 + QC],
                        pvs[hh][0:HD, :], bden[:],
                    )

            def emit_proj(qt):
                q0 = qt * QC
                for et in range(8):
                    ps = ps_pv.tile([P, QC], F32, tag="pv", name=f"yp{qt}{et}")
                    for i in range(2):
                        nc.tensor.matmul(
                            ps[:],
                            wp_sb[i][:, et * P:(et + 1) * P],
                            ot_sb[i][:, q0:q0 + QC],
                            start=(i == 0),
                            stop=(i == 1),
                        )
                    ystage = small.tile([P, QC], F32, tag="ys", name=f"ys{qt}{et}")
                    if et % 2 == 0:
                        nc.scalar.copy(ystage[:], ps[:])
                    else:
                        nc.vector.tensor_copy(ystage[:], ps[:])
                    nc.sync.dma_start(yt[et * P:(et + 1) * P, q0:q0 + QC], ystage[:])

            # software-pipelined emission: proj(qt) goes into the middle of
            # attention(qt+1) so the PE stream never head-of-line blocks on
            # the DVE normalize chain; V tiles trickle in between.
            emit_pair(0, 0)
            for st in range(4, 8):
                emit_v(st)
            emit_pair(0, 1)
            for st in range(8, 12):
                emit_v(st)
            emit_pair(1, 0)
            emit_proj(0)
            for st in range(12, 16):
                emit_v(st)
            emit_pair(1, 1)
            emit_pair(2, 0)
            emit_proj(1)
            emit_pair(2, 1)
            emit_pair(3, 0)
            emit_proj(2)
            emit_pair(3, 1)
            emit_proj(3)

            small_cm.__exit__(None, None, None)
            work_cm.__exit__(None, None, None)
            xtp_cm.__exit__(None, None, None)

    nc.compile()
    return nc


def _shard_inputs(x, w_qkv, b_qkv, w_proj):
    """Build the per-core input maps."""
    in_maps = []
    ones_row = np.ones((1, S), np.float32)
    kk = np.arange(P)[:, None]
    qq = np.arange(QC)[None, :]
    if ATT_BF16:
        import ml_dtypes
        mdt = ml_dtypes.bfloat16
    else:
        mdt = np.float32
    masks_np = np.concatenate(
        sum([[(qq >= kk + P * j).astype(mdt)] * 2 for j in range(4)], []), axis=1
    )
    for c in range(NCORES):
        b, g = divmod(c, 4)
        e0 = g * HPC * HD  # 256*g
        xt_np = np.concatenate([np.ascontiguousarray(x[b].T), ones_row], 0)
        q_rows = w_qkv[e0:e0 + HPC * HD]            # [256, 1024]
        k_rows = w_qkv[D + e0:D + e0 + HPC * HD]
        wqk_np = np.concatenate([q_rows.T, k_rows.T], 1)  # [1024, 512]
        wv_np = np.zeros((D + 1, HPC * (HD + 1)), np.float32)
        for h in range(HPC):
            rows = 2 * D + e0 + h * HD
            wv_np[:D, h * (HD + 1):h * (HD + 1) + HD] = w_qkv[rows:rows + HD].T
            wv_np[D, h * (HD + 1):h * (HD + 1) + HD] = b_qkv[rows:rows + HD]
            wv_np[D, h * (HD + 1) + HD] = 1.0
        bqk_np = np.stack(
            [b_qkv[e0:e0 + P], b_qkv[e0 + P:e0 + 2 * P],
             b_qkv[D + e0:D + e0 + P], b_qkv[D + e0 + P:D + e0 + 2 * P]], 1
        ).astype(np.float32)
        wp_np = np.ascontiguousarray(w_proj[:, e0:e0 + HPC * HD].T)  # [256, 1024]
        in_maps.append({
            "xt": np.ascontiguousarray(xt_np.astype(mdt)),
            "wqk": np.ascontiguousarray(wqk_np.astype(mdt)),
            "wv": wv_np.astype(mdt),
            "bqk": np.ascontiguousarray(bqk_np),
            "masks": masks_np,
            "wp": wp_np,
        })
    return in_maps


def _run(inputs, trace=False, trace_kwargs=None):
    global _PROGRAM
    if _PROGRAM is None:
        _PROGRAM = _build_program()
    nc = _PROGRAM
    x = np.asarray(inputs["x"], np.float32)
    w_qkv = np.asarray(inputs["w_qkv"], np.float32)
    b_qkv = np.asarray(inputs["b_qkv"], np.float32)
    w_proj = np.asarray(inputs["w_proj"], np.float32)
    b_proj = np.asarray(inputs["b_proj"], np.float32)
    in_maps = _shard_inputs(x, w_qkv, b_qkv, w_proj)
    res = run_bass_kernel_spmd(
        nc, in_maps, core_ids=list(range(NCORES)),
        trace=trace, **(trace_kwargs or {}),
    )
    y = np.zeros((B, S, D), np.float32)
    for c in range(NCORES):
        y[c // 4] += res.results[c]["yt"].T
    y += b_proj
    return y, res


def kernel(**inputs):
    y, _ = _run(inputs)
    return y
